# revision 6
# baseline (speedup 1.0000x reference)
"""Edge-parallel graph multi-head attention on 8 Trainium2 NeuronCores.

Strategy (SPMD, zero collectives):
  - Host sorts edges by target node and partitions target nodes into 8
    contiguous block ranges (49 blocks of 128 nodes per core).  Every edge
    lives on the core that owns its target node, so segment-softmax and the
    weighted scatter-sum are core-local and outputs are disjoint row ranges.
  - Per 128-node block, edges are padded to a uniform per-block capacity so
    all cores run an identical program on different data.
  - On chip, per 128-edge tile: gather node_feats[src] (bf16) and a
    precomputed per-edge Q row (f32, by target) with indirect DMA, compute
    K/V with TensorE matmuls (bf16 in, fp32 PSUM accumulate), per-head
    logits + exp on DVE/ACT, then segment-sum via a one-hot matmul
    accumulated in PSUM across the block's tiles.  Softmax normalisation is
    folded into a single divide at block end (exp sums are carried as 8
    extra columns of the scatter matmul).
"""

import sys

if "/opt/trn_rl_repo" not in sys.path:
    sys.path.insert(0, "/opt/trn_rl_repo")

import math
from contextlib import ExitStack

import numpy as np
import ml_dtypes

import concourse.bass as bass
import concourse.bacc as bacc
import concourse.tile as tile
from concourse import mybir
from concourse.masks import make_identity

P = 128
D_NODE = 128
D_EDGE = 64
N_HEADS = 8
HEAD_DIM = 16
N_CORES = 8

F32 = mybir.dt.float32
BF16 = mybir.dt.bfloat16
I32 = mybir.dt.int32

BIG_LID = 1.0e9  # local-id sentinel for padding edges: never matches iota 0..127


# --------------------------------------------------------------------------
# program construction
# --------------------------------------------------------------------------

def build_program(n_tbl, nown, nblk, cap, has_kv_bias):
    """Build the per-core Bass/Tile program (identical across cores).

    n_tbl: rows of the full node-feature gather table
    nown:  nodes owned per core (nblk * 128)
    nblk:  node blocks per core
    cap:   edge slots per block (multiple of 128)
    """
    T = cap // P
    nc = bacc.Bacc("TRN2", target_bir_lowering=False, debug=False,
                   num_devices=N_CORES)

    nf_bf = nc.dram_tensor("nf_bf", [n_tbl, D_NODE], BF16, kind="ExternalInput").ap()
    nfoT = nc.dram_tensor("nfoT", [P, nown], BF16, kind="ExternalInput").ap()
    efT = nc.dram_tensor("efT", [D_EDGE, nblk * cap], BF16, kind="ExternalInput").ap()
    srcid = nc.dram_tensor("srcid", [P, nblk * T], I32, kind="ExternalInput").ap()
    qidx = nc.dram_tensor("qidx", [P, nblk * T], I32, kind="ExternalInput").ap()
    lid = nc.dram_tensor("lid", [P, nblk * T], F32, kind="ExternalInput").ap()
    wq = nc.dram_tensor("wq", [D_NODE, D_NODE], BF16, kind="ExternalInput").ap()
    wkv0 = nc.dram_tensor("wkv0", [D_NODE, 2 * D_NODE], BF16, kind="ExternalInput").ap()
    wkv1 = nc.dram_tensor("wkv1", [D_EDGE, 2 * D_NODE], BF16, kind="ExternalInput").ap()
    wo = nc.dram_tensor("wo", [D_NODE, D_NODE], BF16, kind="ExternalInput").ap()
    brow_q = nc.dram_tensor("brow_q", [1, D_NODE], BF16, kind="ExternalInput").ap()
    brow_kv = nc.dram_tensor("brow_kv", [1, 2 * D_NODE], BF16, kind="ExternalInput").ap()
    brow_o = nc.dram_tensor("brow_o", [1, D_NODE], BF16, kind="ExternalInput").ap()
    out = nc.dram_tensor("out", [nown, D_NODE], F32, kind="ExternalOutput").ap()

    YW = D_NODE + N_HEADS  # scatter payload width: weighted V ++ exp sums

    with tile.TileContext(nc) as tc, ExitStack() as ctx:
        const = ctx.enter_context(tc.tile_pool(name="const", bufs=1))
        dram = ctx.enter_context(tc.tile_pool(name="dram", bufs=1, space="DRAM"))
        blk = ctx.enter_context(tc.tile_pool(name="blk", bufs=2))
        tp = ctx.enter_context(tc.tile_pool(name="tp", bufs=3))
        ps_t = ctx.enter_context(tc.tile_pool(name="ps_t", bufs=2, space="PSUM"))
        ps_kv = ctx.enter_context(tc.tile_pool(name="ps_kv", bufs=2, space="PSUM"))
        ps_acc = ctx.enter_context(tc.tile_pool(name="ps_acc", bufs=2, space="PSUM"))
        ps_blk = ctx.enter_context(tc.tile_pool(name="ps_blk", bufs=1, space="PSUM"))

        # ---- constants ----
        iota_i = const.tile([P, P], I32)
        nc.gpsimd.iota(iota_i[:], pattern=[[1, P]], base=0, channel_multiplier=0)
        iota_f = const.tile([P, P], F32)
        nc.vector.tensor_copy(iota_f[:], iota_i[:])
        ident = const.tile([P, P], BF16)
        make_identity(nc, ident[:])
        ones_bf = const.tile([1, P], BF16)
        nc.gpsimd.memset(ones_bf[:], 1.0)

        wq_sb = const.tile([D_NODE, D_NODE], BF16)
        nc.sync.dma_start(out=wq_sb[:], in_=wq[:])
        wkv0_sb = const.tile([D_NODE, 2 * D_NODE], BF16)
        nc.sync.dma_start(out=wkv0_sb[:], in_=wkv0[:])
        wkv1_sb = const.tile([D_EDGE, 2 * D_NODE], BF16)
        nc.sync.dma_start(out=wkv1_sb[:], in_=wkv1[:])
        wo_sb = const.tile([D_NODE, D_NODE], BF16)
        nc.sync.dma_start(out=wo_sb[:], in_=wo[:])
        browq_sb = const.tile([1, D_NODE], BF16)
        nc.sync.dma_start(out=browq_sb[:], in_=brow_q[:])
        browkv_sb = const.tile([1, 2 * D_NODE], BF16)
        nc.sync.dma_start(out=browkv_sb[:], in_=brow_kv[:])
        browo_sb = const.tile([1, D_NODE], BF16)
        nc.sync.dma_start(out=browo_sb[:], in_=brow_o[:])

        q_tbl = dram.tile([nown, D_NODE], F32)

        # ---- phase 0: per-owned-node Q table (Q = nf @ Wq + bq) ----
        for b in range(nblk):
            nfoT_b = blk.tile([P, P], BF16, tag="nfoT")
            nc.sync.dma_start(out=nfoT_b[:], in_=nfoT[:, b * P:(b + 1) * P])
            q_ps = ps_blk.tile([P, P], F32, tag="blkps")
            nc.tensor.matmul(q_ps[:], lhsT=nfoT_b[:], rhs=wq_sb[:],
                             start=True, stop=False)
            nc.tensor.matmul(q_ps[:], lhsT=ones_bf[:], rhs=browq_sb[:],
                             start=False, stop=True)
            q_blk = blk.tile([P, P], F32, tag="qblk")
            nc.scalar.copy(q_blk[:], q_ps[:])
            nc.sync.dma_start(out=q_tbl[b * P:(b + 1) * P, :], in_=q_blk[:])

        # ---- phase 1: edge tiles ----
        for b in range(nblk):
            efT_b = blk.tile([D_EDGE, cap], BF16, tag="efT")
            nc.sync.dma_start(out=efT_b[:], in_=efT[:, b * cap:(b + 1) * cap])
            src_b = blk.tile([P, T], I32, tag="src")
            nc.sync.dma_start(out=src_b[:], in_=srcid[:, b * T:(b + 1) * T])
            qidx_b = blk.tile([P, T], I32, tag="qidx")
            nc.sync.dma_start(out=qidx_b[:], in_=qidx[:, b * T:(b + 1) * T])
            lid_b = blk.tile([P, T], F32, tag="lid")
            nc.sync.dma_start(out=lid_b[:], in_=lid[:, b * T:(b + 1) * T])

            acc = ps_acc.tile([P, YW], F32)

            for t in range(T):
                nfsrc = tp.tile([P, D_NODE], BF16, tag="nfsrc")
                nc.gpsimd.indirect_dma_start(
                    out=nfsrc[:], out_offset=None, in_=nf_bf[:],
                    in_offset=bass.IndirectOffsetOnAxis(ap=src_b[:, t:t + 1], axis=0),
                )
                qg = tp.tile([P, D_NODE], F32, tag="qg")
                nc.gpsimd.indirect_dma_start(
                    out=qg[:], out_offset=None, in_=q_tbl[:],
                    in_offset=bass.IndirectOffsetOnAxis(ap=qidx_b[:, t:t + 1], axis=0),
                )

                nfT_ps = ps_t.tile([P, P], BF16)
                nc.tensor.transpose(nfT_ps[:], nfsrc[:], ident[:])
                nfT = tp.tile([P, P], BF16, tag="nfT")
                nc.scalar.copy(nfT[:], nfT_ps[:])

                kv = ps_kv.tile([P, 2 * D_NODE], F32)
                nc.tensor.matmul(kv[:], lhsT=nfT[:], rhs=wkv0_sb[:],
                                 start=True, stop=False)
                nc.tensor.matmul(kv[:], lhsT=efT_b[:, t * P:(t + 1) * P],
                                 rhs=wkv1_sb[:], start=False, stop=not has_kv_bias)
                if has_kv_bias:
                    nc.tensor.matmul(kv[:], lhsT=ones_bf[:], rhs=browkv_sb[:],
                                     start=False, stop=True)

                S = tp.tile([P, P], BF16, tag="S")
                nc.gpsimd.tensor_scalar(
                    out=S[:], in0=iota_f[:], scalar1=lid_b[:, t:t + 1],
                    scalar2=None, op0=mybir.AluOpType.is_equal,
                )

                k_sb = tp.tile([P, D_NODE], F32, tag="k")
                nc.scalar.copy(k_sb[:], kv[:, 0:D_NODE])
                qk = tp.tile([P, D_NODE], F32, tag="qk")
                nc.vector.tensor_tensor(out=qk[:], in0=qg[:], in1=k_sb[:],
                                        op=mybir.AluOpType.mult)
                a8 = tp.tile([P, N_HEADS], F32, tag="a8")
                nc.vector.tensor_reduce(
                    out=a8[:],
                    in_=qk[:].rearrange("p (h c) -> p h c", c=HEAD_DIM),
                    axis=mybir.AxisListType.X, op=mybir.AluOpType.add,
                )
                e8 = tp.tile([P, N_HEADS], F32, tag="e8")
                nc.scalar.activation(e8[:], a8[:],
                                     mybir.ActivationFunctionType.Exp,
                                     scale=1.0 / math.sqrt(HEAD_DIM))

                Y = tp.tile([P, YW], BF16, tag="Y")
                nc.vector.tensor_tensor(
                    out=Y[:, 0:D_NODE].rearrange("p (h c) -> p h c", c=HEAD_DIM),
                    in0=kv[:, D_NODE:2 * D_NODE].rearrange("p (h c) -> p h c", c=HEAD_DIM),
                    in1=e8[:, :, None].to_broadcast([P, N_HEADS, HEAD_DIM]),
                    op=mybir.AluOpType.mult,
                )
                nc.vector.tensor_copy(Y[:, D_NODE:YW], e8[:])

                nc.tensor.matmul(acc[:], lhsT=S[:], rhs=Y[:],
                                 start=(t == 0), stop=(t == T - 1))

            # ---- block epilogue: normalise + output projection ----
            s1 = blk.tile([P, N_HEADS], F32, tag="s1")
            nc.vector.tensor_scalar(out=s1[:], in0=acc[:, D_NODE:YW],
                                    scalar1=1.0e-30, scalar2=None,
                                    op0=mybir.AluOpType.max)
            r8 = blk.tile([P, N_HEADS], F32, tag="r8")
            nc.vector.reciprocal(r8[:], s1[:])
            o_bf = blk.tile([P, D_NODE], BF16, tag="obf")
            nc.vector.tensor_tensor(
                out=o_bf[:].rearrange("p (h c) -> p h c", c=HEAD_DIM),
                in0=acc[:, 0:D_NODE].rearrange("p (h c) -> p h c", c=HEAD_DIM),
                in1=r8[:, :, None].to_broadcast([P, N_HEADS, HEAD_DIM]),
                op=mybir.AluOpType.mult,
            )
            oT_ps = ps_blk.tile([P, P], BF16, tag="blkps_t")
            nc.tensor.transpose(oT_ps[:], o_bf[:], ident[:])
            oT = blk.tile([P, P], BF16, tag="oT")
            nc.scalar.copy(oT[:], oT_ps[:])
            out_ps = ps_blk.tile([P, P], F32, tag="blkps")
            nc.tensor.matmul(out_ps[:], lhsT=oT[:], rhs=wo_sb[:],
                             start=True, stop=False)
            nc.tensor.matmul(out_ps[:], lhsT=ones_bf[:], rhs=browo_sb[:],
                             start=False, stop=True)
            out_sb = blk.tile([P, D_NODE], F32, tag="outsb")
            nc.scalar.copy(out_sb[:], out_ps[:])
            nc.sync.dma_start(out=out[b * P:(b + 1) * P, :], in_=out_sb[:])

    nc.compile()
    return nc


# --------------------------------------------------------------------------
# host-side sharding / layout
# --------------------------------------------------------------------------

def prepare(node_feats, edge_feats, edge_index, Wq, bq, Wk, bk, Wv, bv, Wo, bo,
            n_cores=N_CORES):
    node_feats = np.asarray(node_feats, dtype=np.float32)
    edge_feats = np.asarray(edge_feats, dtype=np.float32)
    ei = np.asarray(edge_index)
    src = ei[0].astype(np.int64)
    tgt = ei[1].astype(np.int64)

    n = node_feats.shape[0]
    e = edge_feats.shape[0]
    nbt = -(-n // P)                      # total node blocks (ceil)
    nbt = -(-nbt // n_cores) * n_cores    # pad to a multiple of n_cores
    nblk = nbt // n_cores
    nown = nblk * P

    blk_of_edge = tgt >> 7
    counts = np.bincount(blk_of_edge, minlength=nbt)
    cap = max(P, int(-(-counts.max() // P) * P))
    T = cap // P

    # stable sort edges by target block, then slot them per block
    order = np.argsort(tgt, kind="stable")
    starts = np.zeros(nbt + 1, dtype=np.int64)
    np.cumsum(counts, out=starts[1:])

    # slot id within the global padded [nbt, cap] edge layout
    slot = np.empty(e, dtype=np.int64)
    # edges in `order` are grouped by block; position within block:
    pos_in_blk = np.arange(e, dtype=np.int64) - starts[blk_of_edge[order]]
    slot[order] = blk_of_edge[order] * cap + pos_in_blk

    # per-slot tables (global, then reshaped per core)
    src_slots = np.zeros(nbt * cap, dtype=np.int32)
    lid_slots = np.full(nbt * cap, BIG_LID, dtype=np.float32)
    qidx_slots = np.zeros(nbt * cap, dtype=np.int32)
    ef_slots = np.zeros((nbt * cap, D_EDGE), dtype=ml_dtypes.bfloat16)

    src_slots[slot] = src.astype(np.int32)
    lid_slots[slot] = (tgt & 127).astype(np.float32)
    # q-table row: node index local to the owning core
    qidx_slots[slot] = ((tgt >> 7) % nblk * P + (tgt & 127)).astype(np.int32)
    ef_slots[slot] = edge_feats.astype(ml_dtypes.bfloat16)

    nf_bf = node_feats.astype(ml_dtypes.bfloat16)

    Wk = np.asarray(Wk, np.float32)
    Wv = np.asarray(Wv, np.float32)
    wkv0 = np.concatenate([Wk[:D_NODE], Wv[:D_NODE]], axis=1).astype(ml_dtypes.bfloat16)
    wkv1 = np.concatenate([Wk[D_NODE:], Wv[D_NODE:]], axis=1).astype(ml_dtypes.bfloat16)
    wq_b = np.asarray(Wq, np.float32).astype(ml_dtypes.bfloat16)
    wo_b = np.asarray(Wo, np.float32).astype(ml_dtypes.bfloat16)
    brow_q = np.asarray(bq, np.float32)[None, :].astype(ml_dtypes.bfloat16)
    brow_kv = np.concatenate([np.asarray(bk, np.float32),
                              np.asarray(bv, np.float32)])[None, :].astype(ml_dtypes.bfloat16)
    brow_o = np.asarray(bo, np.float32)[None, :].astype(ml_dtypes.bfloat16)
    has_kv_bias = bool(np.any(np.asarray(bk) != 0) or np.any(np.asarray(bv) != 0))

    # padded node features, transposed per core
    nf_pad = np.zeros((nbt * P, D_NODE), dtype=np.float32)
    nf_pad[:n] = node_feats

    in_maps = []
    for c in range(n_cores):
        b0 = c * nblk
        sl_e = slice(b0 * cap, (b0 + nblk) * cap)
        sl_t = slice(b0 * cap, (b0 + nblk) * cap)
        # [nblk*cap] -> [nblk, T, P] -> [P, nblk*T]
        def col(x, dt):
            return np.ascontiguousarray(
                x[sl_e].reshape(nblk * T, P).T).astype(dt)
        in_maps.append({
            "nf_bf": nf_bf,
            "nfoT": np.ascontiguousarray(
                nf_pad[b0 * P:(b0 + nblk) * P].T).astype(ml_dtypes.bfloat16),
            "efT": np.ascontiguousarray(ef_slots[sl_e].T),
            "srcid": col(src_slots, np.int32),
            "qidx": col(qidx_slots, np.int32),
            "lid": col(lid_slots, np.float32),
            "wq": wq_b, "wkv0": wkv0, "wkv1": wkv1, "wo": wo_b,
            "brow_q": brow_q, "brow_kv": brow_kv, "brow_o": brow_o,
        })

    meta = dict(n=n, nown=nown, nblk=nblk, cap=cap, n_tbl=n,
                has_kv_bias=has_kv_bias, n_cores=n_cores)
    return in_maps, meta


# --------------------------------------------------------------------------
# execution
# --------------------------------------------------------------------------

_PROGRAM_CACHE = {}
LAST_RUN = {}


def get_program(meta):
    key = (meta["n_tbl"], meta["nown"], meta["nblk"], meta["cap"],
           meta["has_kv_bias"])
    if key not in _PROGRAM_CACHE:
        _PROGRAM_CACHE[key] = build_program(*key)
    return _PROGRAM_CACHE[key]


def kernel(**inputs):
    from concourse.bass_utils import run_bass_kernel_spmd

    in_maps, meta = prepare(**inputs)
    nc = get_program(meta)
    res = run_bass_kernel_spmd(nc, in_maps, list(range(meta["n_cores"])))
    LAST_RUN["nc"] = nc
    LAST_RUN["in_maps"] = in_maps
    LAST_RUN["meta"] = meta
    LAST_RUN["res"] = res
    full = np.concatenate([res.results[i]["out"] for i in range(meta["n_cores"])],
                          axis=0)
    return np.ascontiguousarray(full[:meta["n"]]).astype(np.float32)
